# revision 2
# baseline (speedup 1.0000x reference)
"""Trainium2 Bass kernel for nn_Base_55954833932808 (GNN message passing).

Distribution (8 NeuronCores):
  - Data-parallel over graphs: core c owns node rows [2048c, 2048c+2048)
    (graphs [8c, 8c+8)); conv weights replicated.
  - Node-head weights sharded along the node-position axis: core c owns
    positions [32c, 32c+32) for all 64 graphs (expert-parallel).
  - segment_sum(x[src], dst) is computed per dst-shard: host sorts edges by
    dst, pads each 128-node block to T*128 edges; the device gathers x[src]
    rows (dma_gather, bf16) and reduces them on the TensorEngine with
    per-tile 0/1 selection matrices accumulated in PSUM.
  - BatchNorm stats are AllReduced (4KB); updated x is AllGathered (bf16)
    each layer so every core can gather arbitrary src rows.
"""

import numpy as np
import ml_dtypes

import concourse.bass as bass
import concourse.mybir as mybir
import concourse.tile as tile
from concourse import bacc
from concourse.masks import make_identity
from concourse.bass_utils import run_bass_kernel_spmd

# ---------------- problem constants (hardcoded per contest rules) -------------
NUM_NODES = 256
B = 64                  # graphs
N = NUM_NODES * B       # 16384 nodes
E = N * 16              # 262144 edges
HID = 512
L = 3
DSH = 512
GH1, GH2 = 512, 256
GOUT = 8
NH1, NH2 = 256, 128
EPS = 1e-5

NC = 8                  # cores
NPC = N // NC           # 2048 nodes per core
NB = 16                 # 128-node blocks per core
P = 128
KC = HID // P           # 4 feature chunks
MC = NPC // P           # 16 node chunks per core
POS_PC = NUM_NODES // NC  # 32 node-head positions per core

BF16 = mybir.dt.bfloat16
F32 = mybir.dt.float32
I16 = mybir.dt.int16

_nbf = ml_dtypes.bfloat16


def _rowperm():
    """node id n -> rank-major row r = (c, p, m)."""
    n = np.arange(N)
    c = n // NPC
    loc = n % NPC
    m = loc // P
    p = loc % P
    return c * NPC + p * MC + m


_ROWPERM = _rowperm()          # r[n]: where node n lives in permuted layout
_INVPERM = np.argsort(_ROWPERM)


def _wrap_idx(idx, width=None):
    """Wrap an index list into the dma_gather [128, n/16] layout
    (first 16 partitions hold the indices; replicated to all 8 Q7 groups)."""
    n = len(idx)
    assert n % 16 == 0
    w = n // 16
    if width is None:
        width = w
    out = np.zeros((128, width), np.int16)
    blk = np.asarray(idx, np.int16).reshape(w, 16).T  # [16, w]
    for r in range(8):
        out[16 * r:16 * r + 16, :w] = blk
    return out


# ------------------------------ host preprocessing ---------------------------

def _host_prep(inputs):
    x = np.asarray(inputs["x"], np.float32)
    ei = np.asarray(inputs["edge_index"], np.int64)
    src, dst = ei[0], ei[1]

    xp = x[_INVPERM]                       # permuted rows: row r holds node _INVPERM[r]
    x_pm = xp.astype(_nbf)                 # [N, HID] bf16, rank-major layout

    # --- per-core edge structures ---
    order = np.argsort(dst, kind="stable")
    src_s, dst_s = src[order], dst[order]
    core_of = dst_s // NPC

    per_core = []
    counts = np.zeros((NC, NB), np.int64)
    for c in range(NC):
        sel = core_of == c
        s_c, d_c = src_s[sel], dst_s[sel] - c * NPC
        blocks = []
        for b in range(NB):
            bs = d_c // P == b
            blocks.append((s_c[bs], d_c[bs] - b * P))
            counts[c, b] = bs.sum()
        per_core.append(blocks)

    TB = tuple(int(np.ceil(counts[:, b].max() / P)) for b in range(NB))
    OFF = np.concatenate([[0], np.cumsum(TB)])  # tile offsets per block
    TT = int(OFF[-1])

    S_all, idx_all = [], []
    for c in range(NC):
        S = np.zeros((128, TT * P), _nbf)
        idxw = np.zeros((128, TT * 8), np.int16)
        for b in range(NB):
            s_b, n_b = per_core[c][b]
            cnt = len(s_b)
            TW = TB[b] * P
            pad_idx = np.zeros(TW, np.int64)
            pad_idx[:cnt] = _ROWPERM[s_b]   # remapped gather rows; pads -> row 0
            idxw[:, OFF[b] * 8:OFF[b + 1] * 8] = _wrap_idx(pad_idx, TW // 16)
            tt = np.arange(cnt) // P
            pp = np.arange(cnt) % P
            S[pp, (int(OFF[b]) + tt) * P + n_b] = 1.0
        S_all.append(S)
        idx_all.append(idxw)

    # --- weights ---
    def chunked(w):  # [Kin, F] -> [128, (Kin/128)*F]
        ki, f = w.shape
        return np.ascontiguousarray(
            w.reshape(ki // P, P, f).transpose(1, 0, 2).reshape(P, -1)).astype(_nbf)

    wcat = np.zeros((L, P, 2 * KC * HID), _nbf)
    for l in range(L):
        wself = np.asarray(inputs["conv_wself"][l], np.float32)
        wnei = np.asarray(inputs["conv_wneigh"][l], np.float32)
        wcat[l] = chunked(np.concatenate([wself, wnei], axis=0))

    gs_w1 = chunked(np.asarray(inputs["gs_w1"], np.float32) / NUM_NODES)
    gs_w2 = chunked(np.asarray(inputs["gs_w2"], np.float32))
    gh_w1 = chunked(np.asarray(inputs["gh_w1"], np.float32))
    gh_w2 = chunked(np.asarray(inputs["gh_w2"], np.float32))
    gh_w3 = chunked(np.asarray(inputs["gh_w3"], np.float32))

    def pcol(b):  # [F] -> [128, F/128] f32 (per-partition bias columns)
        return np.ascontiguousarray(
            np.asarray(b, np.float32).reshape(-1, P).T)

    gs_b1 = pcol(inputs["gs_b1"]); gs_b2 = pcol(inputs["gs_b2"])
    gh_b1 = pcol(inputs["gh_b1"]); gh_b2 = pcol(inputs["gh_b2"])
    gh_b3 = np.asarray(inputs["gh_b3"], np.float32).reshape(GOUT, 1)

    pool_ind = np.zeros((P, NB * 8), _nbf)
    for m in range(NB):
        pool_ind[:, m * 8 + m // 2] = 1.0

    bn_g = np.asarray(inputs["bn_gamma"], np.float32)
    bn_b = np.asarray(inputs["bn_beta"], np.float32)

    # --- node-head (per core) ---
    nh_w1 = np.asarray(inputs["nh_w1"], np.float32)   # [256, 512, 256]
    nh_w2 = np.asarray(inputs["nh_w2"], np.float32)   # [256, 256, 128]
    nh_w3 = np.asarray(inputs["nh_w3"], np.float32)   # [256, 128, 1]
    nh_b1 = np.asarray(inputs["nh_b1"], np.float32)
    nh_b2 = np.asarray(inputs["nh_b2"], np.float32)
    nh_b3 = np.asarray(inputs["nh_b3"], np.float32)

    per_core_maps = []
    for c in range(NC):
        pos = np.arange(POS_PC) + POS_PC * c
        w1 = np.stack([chunked(nh_w1[p]) for p in pos])          # [32,128,4*256]
        w2 = np.stack([chunked(nh_w2[p]) for p in pos])          # [32,128,2*128]
        w3 = np.ascontiguousarray(nh_w3[pos, :, 0].T).astype(_nbf)  # [128, 32]
        b1 = np.zeros((P, POS_PC * 2), np.float32)
        for j in range(POS_PC):
            b1[:, 2 * j] = nh_b1[pos[j], :P]
            b1[:, 2 * j + 1] = nh_b1[pos[j], P:]
        b2 = np.ascontiguousarray(nh_b2[pos].T)                  # [128, 32]
        b3 = nh_b3[pos].reshape(POS_PC, 1)

        # node-head gather: 16 ops x 128 idx (2 positions x 64 graphs)
        nhidx = np.zeros((128, 16 * 8), np.int16)
        for jj in range(16):
            ids = []
            for j in (2 * jj, 2 * jj + 1):
                g = np.arange(B)
                ids.append(_ROWPERM[g * NUM_NODES + pos[j]])
            nhidx[:, jj * 8:(jj + 1) * 8] = _wrap_idx(np.concatenate(ids), 8)

        # own-node transpose-gather indices (x.T tiles), i = m*128+q
        own = np.arange(NPC) + c * NPC
        own_nodes = own  # node ids in layout order m*128+q? build explicitly:
        mm = np.arange(NPC) // P
        qq = np.arange(NPC) % P
        own_nodes = c * NPC + mm * P + qq
        xt_idx = _wrap_idx(_ROWPERM[own_nodes], NPC // 16)

        per_core_maps.append(dict(
            x_pm=x_pm,
            x_slice=np.ascontiguousarray(x_pm[c * NPC:(c + 1) * NPC]),
            S=S_all[c], eidx=idx_all[c], xt_idx=xt_idx,
            wcat=wcat, bn_g=bn_g, bn_b=bn_b,
            gs_w1=gs_w1, gs_w2=gs_w2, gh_w1=gh_w1, gh_w2=gh_w2, gh_w3=gh_w3,
            gs_b1=gs_b1, gs_b2=gs_b2, gh_b1=gh_b1, gh_b2=gh_b2, gh_b3=gh_b3,
            pool_ind=pool_ind,
            nh_w1=w1, nh_w2=w2, nh_w3=w3, nh_b1=b1, nh_b2=b2, nh_b3=b3,
            nh_idx=nhidx,
        ))
    return TB, per_core_maps


# ------------------------------ device program --------------------------------

class _StopBuild(Exception):
    pass


def _build_program(TB, debug=False, n_layers=L, use_cc=True, agg_only=False,
                   do_graph=True, do_node=True):
    TB = tuple(TB)
    OFF = [0]
    for t in TB:
        OFF.append(OFF[-1] + t)
    TT = OFF[-1]
    TMAX = max(TB)

    nc = bacc.Bacc("TRN2", target_bir_lowering=False, debug=False, num_devices=NC)

    dt = {}
    def din(name, shape, dtype):
        dt[name] = nc.dram_tensor(name, list(shape), dtype, kind="ExternalInput")
        return dt[name]

    din("x_pm", (N, HID), BF16)
    din("S", (P, TT * P), BF16)
    din("eidx", (P, TT * 8), I16)
    din("xt_idx", (P, NPC // 16), I16)
    din("wcat", (L, P, 2 * KC * HID), BF16)
    din("bn_g", (L, HID), F32)
    din("bn_b", (L, HID), F32)
    din("gs_w1", (P, KC * DSH), BF16)
    din("gs_w2", (P, KC * DSH), BF16)
    din("gh_w1", (P, KC * GH1), BF16)
    din("gh_w2", (P, KC * GH2), BF16)
    din("gh_w3", (P, 2 * GOUT), BF16)
    din("gs_b1", (P, KC), F32)
    din("gs_b2", (P, KC), F32)
    din("gh_b1", (P, KC), F32)
    din("gh_b2", (P, GH2 // P), F32)
    din("gh_b3", (GOUT, 1), F32)
    din("pool_ind", (P, NB * 8), BF16)
    din("nh_w1", (POS_PC, P, KC * NH1), BF16)
    din("nh_w2", (POS_PC, P, 2 * NH2), BF16)
    din("nh_w3", (P, POS_PC), BF16)
    din("nh_b1", (P, POS_PC * 2), F32)
    din("nh_b2", (P, POS_PC), F32)
    din("nh_b3", (POS_PC, 1), F32)
    din("nh_idx", (P, 16 * 8), I16)

    out_node = nc.dram_tensor("out_node", [POS_PC, B], F32, kind="ExternalOutput")
    out_graph = nc.dram_tensor("out_graph", [GOUT, B], F32, kind="ExternalOutput")
    dbg = {}
    if debug:
        for nm in ("dbg_agg", "dbg_h"):
            dbg[nm] = nc.dram_tensor(nm, [P, MC * HID], F32, kind="ExternalOutput")
        for nm in ("dbg_x1", "dbg_x3"):
            dbg[nm] = nc.dram_tensor(nm, [P, MC * HID], BF16, kind="ExternalOutput")
        dbg["dbg_stat"] = nc.dram_tensor("dbg_stat", [1, 2 * HID], F32,
                                         kind="ExternalOutput")

    # collective buffers (internal DRAM, Shared address space)
    st_in_d, st_out_d, ag_in_d, xrep_d = [], [], [], []
    for l in range(n_layers):
        st_in_d.append(nc.dram_tensor(f"st_in{l}", [1, 2 * HID], F32))
        st_out_d.append(nc.dram_tensor(f"st_out{l}", [1, 2 * HID], F32,
                                       addr_space="Shared"))
        ag_in_d.append(nc.dram_tensor(f"ag_in{l}", [NPC, HID], BF16))
        xrep_d.append(nc.dram_tensor(f"xrep{l}", [N, HID], BF16,
                                     addr_space="Shared"))

    groups = [list(range(NC))]
    Relu = mybir.ActivationFunctionType.Relu
    Copy = mybir.ActivationFunctionType.Copy
    Ident = mybir.ActivationFunctionType.Identity
    Square = mybir.ActivationFunctionType.Square
    Sqrt = mybir.ActivationFunctionType.Sqrt
    ADD = mybir.AluOpType.add
    MULT = mybir.AluOpType.mult
    SUB = mybir.AluOpType.subtract

    from contextlib import ExitStack
    with tile.TileContext(nc) as tc, ExitStack() as octx:
        pp = octx.enter_context(tc.tile_pool(name="outer", bufs=1))
        xp = octx.enter_context(tc.tile_pool(name="xnode", bufs=1))
        psA = octx.enter_context(tc.tile_pool(name="psA", bufs=2, space="PSUM"))
        psB = octx.enter_context(tc.tile_pool(name="psB", bufs=2, space="PSUM"))
        psS = octx.enter_context(tc.tile_pool(name="psS", bufs=1, space="PSUM"))
        psT = octx.enter_context(tc.tile_pool(name="psT", bufs=2, space="PSUM"))

        # const APs for activation biases
        zero_c = pp.tile([P, 1], F32, tag="zeroc")
        nc.vector.memset(zero_c[:], 0.0)
        nc.const_aps.aps[(F32, 0.0)] = zero_c[:]
        eps_c = pp.tile([P, 1], F32, tag="epsc")
        nc.vector.memset(eps_c[:], EPS)
        nc.const_aps.aps[(F32, EPS)] = eps_c[:]
        ones_sb = pp.tile([P, 1], BF16, tag="ones")
        nc.vector.memset(ones_sb[:], 1.0)
        ident = pp.tile([P, P], BF16, tag="ident")
        make_identity(nc, ident[:])

        x_cur = None
        xrep_prev = None

        with ExitStack() as lctx:
            lp = lctx.enter_context(tc.tile_pool(name="lpersist", bufs=1))
            wp = lctx.enter_context(tc.tile_pool(name="wpool", bufs=1))
            xtp = lctx.enter_context(tc.tile_pool(name="xtp", bufs=1))
            gp = lctx.enter_context(tc.tile_pool(name="gpool", bufs=3))
            hp = lctx.enter_context(tc.tile_pool(name="hpool", bufs=1))
            wk = lctx.enter_context(tc.tile_pool(name="work", bufs=3))
            sm4 = lctx.enter_context(tc.tile_pool(name="v4", bufs=4))
            sm2 = lctx.enter_context(tc.tile_pool(name="v2", bufs=2))
            smr = lctx.enter_context(tc.tile_pool(name="strep", bufs=1))

            S_sb = lp.tile([P, TT, P], BF16, tag="S")
            nc.sync.dma_start(S_sb[:],
                              dt["S"][:].rearrange("p (t n) -> p t n", t=TT))
            eidx_sb = lp.tile([P, TT * 8], I16, tag="eidx")
            nc.sync.dma_start(eidx_sb[:], dt["eidx"][:])
            xtidx_sb = lp.tile([P, NPC // 16], I16, tag="xtidx")
            nc.sync.dma_start(xtidx_sb[:], dt["xt_idx"][:])

            for l in range(n_layers):
                gsrc = dt["x_pm"][:] if xrep_prev is None else xrep_prev[:]

                w_sb = wp.tile([P, 2 * KC, HID], BF16, tag="wcat")
                nc.sync.dma_start(
                    w_sb[:], dt["wcat"][l].rearrange("p (k f) -> p k f",
                                                     k=2 * KC))

                # x.T tiles via transpose-mode gather of own rows
                xT = xtp.tile([P, KC, NPC], BF16, tag="xT")
                nc.gpsimd.dma_gather(xT[:], gsrc, xtidx_sb[:], NPC, NPC, HID,
                                     transpose=True, single_packet=False)

                h_sb = hp.tile([P, MC, HID], BF16, tag="h")

                st_sum = psS.tile([1, HID], F32, space="PSUM", tag="stsum")
                st_sq = psS.tile([1, HID], F32, space="PSUM", tag="stsq")

                for m in range(NB):
                    T_b = TB[m]
                    agg_ps = psA.tile([P, HID], F32, space="PSUM", tag="agg")
                    # gather G for block m in <=1024-idx chunks, matmul with S
                    n1 = (T_b + 1) // 2
                    t0 = 0
                    for piece, tcnt in enumerate([n1, T_b - n1]):
                        if tcnt == 0:
                            continue
                        G = gp.tile([P, (TMAX + 1) // 2, HID], BF16, tag="G")
                        nc.gpsimd.dma_gather(
                            G[:, :tcnt, :], gsrc,
                            eidx_sb[:, (OFF[m] + t0) * 8:(OFF[m] + t0 + tcnt) * 8],
                            tcnt * P, tcnt * P, HID, single_packet=False)
                        for t in range(tcnt):
                            nc.tensor.matmul(
                                agg_ps[:], S_sb[:, OFF[m] + t0 + t, :],
                                G[:, t, :], start=(t0 + t == 0),
                                stop=(t0 + t == T_b - 1))
                        t0 += tcnt
                    agg_sb = wk.tile([P, HID], BF16, tag="aggsb")
                    nc.scalar.activation(agg_sb[:], agg_ps[:], Copy)
                    if debug and l == 0:
                        agg32 = wk.tile([P, HID], F32, tag="aggdbg")
                        nc.vector.tensor_copy(agg32[:], agg_ps[:])
                        nc.sync.dma_start(
                            dbg["dbg_agg"][:, m * HID:(m + 1) * HID], agg32[:])
                    if agg_only:
                        continue

                    # transpose agg (PE, via identity)
                    aggT = wk.tile([P, KC, P], BF16, tag="aggT")
                    for k in range(KC):
                        tr_ps = psT.tile([P, P], BF16, space="PSUM", tag="tr")
                        nc.tensor.transpose(tr_ps[:],
                                            agg_sb[:, k * P:(k + 1) * P],
                                            ident[:])
                        nc.scalar.activation(aggT[:, k, :], tr_ps[:], Copy)

                    # conv: h[m] = [x|agg] @ wcat
                    h_ps = psB.tile([P, HID], F32, space="PSUM", tag="conv")
                    for k in range(KC):
                        nc.tensor.matmul(h_ps[:], xT[:, k, m * P:(m + 1) * P],
                                         w_sb[:, k, :], start=(k == 0),
                                         stop=False)
                    for k in range(KC):
                        nc.tensor.matmul(h_ps[:], aggT[:, k, :],
                                         w_sb[:, KC + k, :], start=False,
                                         stop=(k == KC - 1))
                    nc.vector.tensor_copy(h_sb[:, m, :], h_ps[:])
                    if debug and l == 0:
                        h32 = wk.tile([P, HID], F32, tag="hdbg")
                        nc.vector.tensor_copy(h32[:], h_ps[:])
                        nc.sync.dma_start(
                            dbg["dbg_h"][:, m * HID:(m + 1) * HID], h32[:])

                    hsq = wk.tile([P, HID], BF16, tag="hsq")
                    nc.scalar.activation(hsq[:], h_ps[:], Square)
                    nc.tensor.matmul(st_sum[:], ones_sb[:], h_sb[:, m, :],
                                     start=(m == 0), stop=(m == NB - 1))
                    nc.tensor.matmul(st_sq[:], ones_sb[:], hsq[:],
                                     start=(m == 0), stop=(m == NB - 1))

                if agg_only:
                    break

                # ---- BN stats (+allreduce) ----
                stat_sb = sm2.tile([1, 2 * HID], F32, tag="v2h")
                nc.vector.tensor_copy(stat_sb[:, :HID], st_sum[:])
                nc.vector.tensor_copy(stat_sb[:, HID:], st_sq[:])
                if use_cc:
                    nc.sync.dma_start(st_in_d[l][:], stat_sb[:])
                    nc.gpsimd.collective_compute(
                        "AllReduce", ADD, replica_groups=groups,
                        ins=[st_in_d[l][:].opt()], outs=[st_out_d[l][:].opt()])
                    stat_r = sm2.tile([1, 2 * HID], F32, tag="v2h")
                    nc.sync.dma_start(stat_r[:], st_out_d[l][:])
                else:
                    stat_r = stat_sb
                if debug and l == 0:
                    nc.sync.dma_start(dbg["dbg_stat"][:], stat_r[:])

                # s = gamma / sqrt(var+eps); t = beta - mean*s   (on [1,512])
                mean = sm4.tile([1, HID], F32, tag="v1h")
                nc.vector.tensor_scalar_mul(mean[:], stat_r[:, :HID], 1.0 / N)
                m2t = sm4.tile([1, HID], F32, tag="v1h")
                nc.vector.tensor_tensor(m2t[:], mean[:], mean[:], MULT)
                var = sm4.tile([1, HID], F32, tag="v1h")
                nc.vector.tensor_scalar(var[:], stat_r[:, HID:], 1.0 / N, None,
                                        MULT)
                nc.vector.tensor_tensor(var[:], var[:], m2t[:], SUB)
                std = sm4.tile([1, HID], F32, tag="v1h")
                nc.scalar.activation(std[:], var[:], Sqrt, bias=EPS)
                inv = sm4.tile([1, HID], F32, tag="v1h")
                nc.vector.reciprocal(inv[:], std[:])
                gam = sm4.tile([1, HID], F32, tag="v1h")
                nc.sync.dma_start(gam[:], dt["bn_g"][l][None, :])
                st_pack = sm2.tile([1, 2 * HID], F32, tag="v2h")
                nc.vector.tensor_tensor(st_pack[:, :HID], gam[:], inv[:], MULT)
                ms = sm4.tile([1, HID], F32, tag="v1h")
                nc.vector.tensor_tensor(ms[:], mean[:], st_pack[:, :HID], MULT)
                bet = sm4.tile([1, HID], F32, tag="v1h")
                nc.sync.dma_start(bet[:], dt["bn_b"][l][None, :])
                nc.vector.tensor_tensor(st_pack[:, HID:], bet[:], ms[:], SUB)
                st_dram = nc.dram_tensor(f"st_pack{l}", [1, 2 * HID], F32)
                nc.sync.dma_start(st_dram[:], st_pack[:])
                st_rep = smr.tile([P, 2 * HID], F32, tag="strep")
                nc.sync.dma_start(st_rep[:],
                                  st_dram[0:1, :].to_broadcast((P, 2 * HID)))

                # ---- normalize + relu -> next x ----
                x_next = xp.tile([P, MC, HID], BF16, tag="xnode")
                for m in range(NB):
                    tmp = wk.tile([P, HID], F32, tag="norm")
                    nc.vector.tensor_tensor(tmp[:], h_sb[:, m, :],
                                            st_rep[:, :HID], MULT)
                    nc.vector.tensor_tensor(tmp[:], tmp[:], st_rep[:, HID:],
                                            ADD)
                    nc.scalar.activation(x_next[:, m, :], tmp[:], Relu)
                x_cur = x_next
                if debug and l == 0:
                    nc.sync.dma_start(dbg["dbg_x1"][:],
                                      x_cur[:].rearrange("p m f -> p (m f)"))

                # ---- allgather x ----
                if use_cc:
                    nc.sync.dma_start(
                        ag_in_d[l][:].rearrange("(pq m) f -> pq (m f)", m=MC),
                        x_cur[:].rearrange("p m f -> p (m f)"))
                    nc.gpsimd.collective_compute(
                        "AllGather", mybir.AluOpType.bypass,
                        replica_groups=groups,
                        ins=[ag_in_d[l][:].opt()], outs=[xrep_d[l][:].opt()])
                    xrep_prev = xrep_d[l]

        if debug and x_cur is not None:
            nc.sync.dma_start(dbg["dbg_x3"][:],
                              x_cur[:].rearrange("p m f -> p (m f)"))

        with ExitStack() as hctx:
            sm = hctx.enter_context(tc.tile_pool(name="once", bufs=1))
            wk2 = hctx.enter_context(tc.tile_pool(name="hwork", bufs=3))

            if not do_graph or x_cur is None:
                gsb0 = sm.tile([GOUT, B], F32, tag="gsb")
                nc.vector.memset(gsb0[:], 0.0)
                nc.sync.dma_start(out_graph[:], gsb0[:])
            else:
                # ------------- graph path (local graphs) -------------
                pind = sm.tile([P, NB, 8], BF16, tag="pind")
                nc.sync.dma_start(
                    pind[:],
                    dt["pool_ind"][:].rearrange("p (m g) -> p m g", g=8))
                xgT = sm.tile([P, KC, 8], BF16, tag="xgT")
                for k in range(KC):
                    pool_ps = psA.tile([P, HID], F32, space="PSUM", tag="agg")
                    for m in range(NB):
                        nc.tensor.matmul(pool_ps[:, :8],
                                         x_cur[:, m, k * P:(k + 1) * P],
                                         pind[:, m, :], start=(m == 0),
                                         stop=(m == NB - 1))
                    nc.scalar.activation(xgT[:, k, :], pool_ps[:, :8], Relu)

                def mlp_layer(src, w_dram, b_dram, kin, kout, act, tag):
                    w_sb2 = sm.tile([P, kin * (kout * P)], BF16, tag=tag + "w")
                    nc.sync.dma_start(w_sb2[:], w_dram[:])
                    w3v = w_sb2[:].rearrange("p (k f) -> p k f", k=kin)
                    b_sb = sm.tile([P, kout], F32, tag=tag + "b")
                    nc.sync.dma_start(b_sb[:], b_dram[:])
                    res = sm.tile([P, kout, 8], BF16, tag=tag + "o")
                    for mo in range(kout):
                        ps = psB.tile([P, HID], F32, space="PSUM", tag="conv")
                        for k in range(kin):
                            nc.tensor.matmul(ps[:, :8],
                                             w3v[:, k, mo * P:(mo + 1) * P],
                                             src[:, k, :], start=(k == 0),
                                             stop=(k == kin - 1))
                        nc.scalar.activation(res[:, mo, :], ps[:, :8], act,
                                             bias=b_sb[:, mo:mo + 1])
                    return res

                z1 = mlp_layer(xgT, dt["gs_w1"], dt["gs_b1"], KC, KC, Ident,
                               "gsw1")
                z2 = mlp_layer(z1, dt["gs_w2"], dt["gs_b2"], KC, KC, Relu,
                               "gsw2")
                g1 = mlp_layer(z2, dt["gh_w1"], dt["gh_b1"], KC, KC, Relu,
                               "ghw1")
                g2 = mlp_layer(g1, dt["gh_w2"], dt["gh_b2"], KC, GH2 // P,
                               Relu, "ghw2")
                w3_sb = sm.tile([P, 2 * GOUT], BF16, tag="ghw3")
                nc.sync.dma_start(w3_sb[:], dt["gh_w3"][:])
                b3_sb = sm.tile([GOUT, 1], F32, tag="ghb3")
                nc.sync.dma_start(b3_sb[:], dt["gh_b3"][:])
                g_ps = psB.tile([P, HID], F32, space="PSUM", tag="conv")
                for k in range(2):
                    nc.tensor.matmul(g_ps[:GOUT, :8],
                                     w3_sb[:, k * GOUT:(k + 1) * GOUT],
                                     g2[:, k, :], start=(k == 0),
                                     stop=(k == 1))
                gsb = sm.tile([GOUT, B], F32, tag="gsb")
                nc.vector.memset(gsb[:], 0.0)
                nc.scalar.activation(gsb[:, :8], g_ps[:GOUT, :8], Ident,
                                     bias=b3_sb[:])
                nc.sync.dma_start(out_graph[:], gsb[:])

            if not do_node or xrep_prev is None and n_layers < L:
                zsb2 = sm.tile([POS_PC, B], F32, tag="nodeS")
                nc.vector.memset(zsb2[:], 0.0)
                nc.sync.dma_start(out_node[:], zsb2[:])
            else:
                # ------------- node head -------------
                nhw3_sb = sm.tile([P, POS_PC], BF16, tag="nhw3")
                nc.sync.dma_start(nhw3_sb[:], dt["nh_w3"][:])
                nhb1_sb = sm.tile([P, POS_PC * 2], F32, tag="nhb1")
                nc.sync.dma_start(nhb1_sb[:], dt["nh_b1"][:])
                nhb2_sb = sm.tile([P, POS_PC], F32, tag="nhb2")
                nc.sync.dma_start(nhb2_sb[:], dt["nh_b2"][:])
                nhb3_sb = sm.tile([POS_PC, 1], F32, tag="nhb3")
                nc.sync.dma_start(nhb3_sb[:], dt["nh_b3"][:])
                nhidx_sb = sm.tile([P, 16 * 8], I16, tag="nhidx")
                nc.sync.dma_start(nhidx_sb[:], dt["nh_idx"][:])

                gsrc3 = (xrep_prev[:] if xrep_prev is not None
                         else dt["x_pm"][:])
                nodeflat = sm.tile([1, POS_PC * B], F32, tag="nodeflat")
                for jj in range(16):
                    xpT = wk2.tile([P, KC, P], BF16, tag="xpT")
                    nc.gpsimd.dma_gather(xpT[:], gsrc3,
                                         nhidx_sb[:, jj * 8:(jj + 1) * 8],
                                         P, P, HID, transpose=True)
                    for half in range(2):
                        j = 2 * jj + half
                        w1_sb = wk2.tile([P, KC, NH1], BF16, tag="nhw1")
                        nc.sync.dma_start(
                            w1_sb[:],
                            dt["nh_w1"][j].rearrange("p (k f) -> p k f", k=KC))
                        w2_sb = wk2.tile([P, 2, NH2], BF16, tag="nhw2")
                        nc.sync.dma_start(
                            w2_sb[:],
                            dt["nh_w2"][j].rearrange("p (k f) -> p k f", k=2))
                        rhs = xpT[:, :, half * B:(half + 1) * B]
                        h1T = wk2.tile([P, 2, B], BF16, tag="h1T")
                        for mo in range(2):
                            ps = psB.tile([P, HID], F32, space="PSUM",
                                          tag="conv")
                            for k in range(KC):
                                nc.tensor.matmul(
                                    ps[:, :B],
                                    w1_sb[:, k, mo * P:(mo + 1) * P],
                                    rhs[:, k, :], start=(k == 0),
                                    stop=(k == KC - 1))
                            nc.scalar.activation(
                                h1T[:, mo, :], ps[:, :B], Relu,
                                bias=nhb1_sb[:, 2 * j + mo:2 * j + mo + 1])
                        ps2 = psB.tile([P, HID], F32, space="PSUM", tag="conv")
                        for k in range(2):
                            nc.tensor.matmul(ps2[:, :B], w2_sb[:, k, :],
                                             h1T[:, k, :], start=(k == 0),
                                             stop=(k == 1))
                        h2T = wk2.tile([P, B], BF16, tag="h2T")
                        nc.scalar.activation(h2T[:], ps2[:, :B], Relu,
                                             bias=nhb2_sb[:, j:j + 1])
                        ps3 = psA.tile([P, HID], F32, space="PSUM", tag="agg")
                        nc.tensor.matmul(ps3[:1, :B], nhw3_sb[:, j:j + 1],
                                         h2T[:], start=True, stop=True)
                        nc.scalar.activation(
                            nodeflat[0:1, j * B:(j + 1) * B], ps3[:1, :B],
                            Copy)
                nflat_d = nc.dram_tensor("nflat_d", [1, POS_PC * B], F32)
                nc.sync.dma_start(nflat_d[:], nodeflat[:])
                nodeT = sm.tile([POS_PC, B], F32, tag="nodeT")
                nc.sync.dma_start(
                    nodeT[:],
                    nflat_d[:].rearrange("o (j g) -> (o j) g", j=POS_PC))
                nodeS = sm.tile([POS_PC, B], F32, tag="nodeS")
                nc.vector.tensor_tensor(
                    nodeS[:], nodeT[:],
                    nhb3_sb[:].to_broadcast((POS_PC, B)), ADD)
                nc.sync.dma_start(out_node[:], nodeS[:])

    nc.compile()
    return nc


_PROG_CACHE = {}


def _get_program(TB, debug=False):
    key = (tuple(TB), debug)
    if key not in _PROG_CACHE:
        _PROG_CACHE[key] = _build_program(TB, debug)
    return _PROG_CACHE[key]


def kernel(**inputs):
    res = _run(inputs, debug=False)
    return _assemble(res)


def _run(inputs, debug=False):
    TB, maps = _host_prep(inputs)
    nc = _get_program(TB, debug)
    return run_bass_kernel_spmd(nc, maps, list(range(NC))).results


def _run_traced(inputs):
    TB, maps = _host_prep(inputs)
    nc = _get_program(TB, False)
    return run_bass_kernel_spmd(nc, maps, list(range(NC)), trace=True,
                                trace_cores=list(range(NC)))


def _assemble(results):
    full = np.empty((B, GOUT + NUM_NODES), np.float32)
    for c in range(NC):
        full[:, GOUT + POS_PC * c: GOUT + POS_PC * (c + 1)] = \
            results[c]["out_node"].T
        full[8 * c:8 * (c + 1), :GOUT] = results[c]["out_graph"][:, :8].T
    return full

